# revision 6
# baseline (speedup 1.0000x reference)
"""Multi-head attention (axis-swapped variant) on 8 Trainium2 NeuronCores, v5.

Schedule engineered for the tile-granular dependency tracker and the
TimelineSim cost model:
- PE warm-up matmuls ramp the pstate while input DMAs stream;
- Q/K t0 projections run as 3 concurrent chunk-gated accumulators;
- all remaining projection / transpose / first-half out-projection work is
  a queue of small units drained one-per-exp-flush so the ACT engine never
  starves;
- ctx follows scores with a 2-head lag (AT triple-buffered);
- softmax normalization fused into ctx evacuation (reciprocal + stride-0
  broadcast multiply);
- out-projection split into two column-halves: heads 0-7 partial (y0) is
  computed and DMA'd during attention, heads 8-15 partial (y1) at the tail;
  the host sums y0 + y1 + bo.
"""

import numpy as np
import ml_dtypes

import concourse.bass as bass
import concourse.mybir as mybir
import concourse.tile as tile
from concourse.bass_utils import run_bass_kernel_spmd

F32 = mybir.dt.float32
BF16 = mybir.dt.bfloat16

EMB = 1024
SEQ = 1024
BATCH = 2
NG = 4
HPG = 16
DH = 16
GCOLS = HPG * DH

SPAN = 1536
NJB = 8

SIZES = [SEQ - 128 * jb for jb in range(NJB)]
# j-blocks grouped so each group totals exactly SPAN (one PSUM scores tile
# and one AT tile per group; no j-block crosses a span boundary)
SPAN_GROUPS = [[1, 3], [0, 4], [2, 5, 6, 7]]
assert all(sum(SIZES[jb] for jb in g) == SPAN for g in SPAN_GROUPS)
SPAN_OF = {}   # jb -> (group idx, offset within group)
ORDER_POS = {}
_pos = 0
for _s, _g in enumerate(SPAN_GROUPS):
    _off = 0
    for _jb in _g:
        SPAN_OF[_jb] = (_s, _off)
        ORDER_POS[_jb] = _pos
        _pos += 1
        _off += SIZES[_jb]


def split_excess_waits(nc, cap=1):
    def fix_block(bb, dummy):
        insts = bb.instructions
        i = 0
        while i < len(insts):
            inst = insts[i]
            si = inst.sync_info
            waits = list(si.on_wait) if si is not None and si.on_wait else []
            if len(waits) > cap:
                eng = nc.engines[inst.engine]
                excess, keep = waits[:-cap], waits[-cap:]
                si.on_wait = keep
                pos = i
                for j in range(0, len(excess), cap):
                    chunk = excess[j : j + cap]
                    ev = eng.wait_ge(dummy, 1)
                    cur_list = nc.cur_bb.bb.instructions
                    assert cur_list[-1] is ev.ins
                    cur_list.pop()
                    ev.ins.sync_info.on_wait = chunk
                    insts.insert(pos, ev.ins)
                    pos += 1
                    i += 1
            i += 1

    with nc.semaphore("waitfix_dummy") as dummy:
        for f in nc.m.functions:
            for bb in f.blocks:
                fix_block(bb, dummy)


def build_nc():
    nc = bass.Bass()
    xT_d = nc.declare_dram_parameter("xT", [8, 128, SEQ], BF16, isOutput=False)
    wq_d = nc.declare_dram_parameter("wq", [2, 128, 8, 128], BF16, isOutput=False)
    wk_d = nc.declare_dram_parameter("wk", [2, 128, 8, 128], BF16, isOutput=False)
    wv_d = nc.declare_dram_parameter("wv", [8, 128, GCOLS], BF16, isOutput=False)
    wo_d = nc.declare_dram_parameter("wo", [2, 128, EMB], BF16, isOutput=False)
    id_d = nc.declare_dram_parameter("ident", [128, 128], BF16, isOutput=False)
    ps_d = nc.declare_dram_parameter("psh", [128, 128], BF16, isOutput=False)
    tri_d = nc.declare_dram_parameter("tri", [128, 128], BF16, isOutput=False)
    y0_d = nc.declare_dram_parameter("y0", [8, 128, EMB], BF16, isOutput=True)
    y1_d = nc.declare_dram_parameter("y1", [8, 128, EMB], BF16, isOutput=True)

    with tile.TileContext(nc) as tc:
        with (
            tc.tile_pool(name="big", bufs=1) as big,
            tc.tile_pool(name="atp", bufs=1) as atp,
            tc.tile_pool(name="yst", bufs=8) as yst,
            tc.tile_pool(name="scp", bufs=2, space="PSUM") as scp,
            tc.tile_pool(name="ctxp", bufs=1, space="PSUM") as ctxp,
        ):
            # ---- SBUF ----
            XTk = [big.tile([128, SEQ], BF16, name=f"xt{k}") for k in range(8)]
            WQt = [big.tile([128, 8, 128], BF16, name=f"wq{t}") for t in range(2)]
            WKt = [big.tile([128, 8, 128], BF16, name=f"wk{t}") for t in range(2)]
            WV = big.tile([128, 8, GCOLS], BF16)
            WO = big.tile([128, 2, EMB], BF16)
            IDT = big.tile([128, 128], BF16)
            PSH = big.tile([128, 128], BF16)
            QTt = [big.tile([128, SEQ], BF16, name=f"qt{t}") for t in range(2)]
            KTt = [big.tile([128, SEQ], BF16, name=f"kt{t}") for t in range(2)]
            QSt = [big.tile([128, SEQ], BF16, name=f"qs{t}") for t in range(2)]
            KSt = [big.tile([128, SEQ], BF16, name=f"ks{t}") for t in range(2)]
            VAr = [big.tile([128, 8, 8, 17], BF16, name=f"va{r}") for r in range(2)]
            CTr = [big.tile([128, 8, 128], BF16, name=f"ct{r}") for r in range(2)]
            CNr = [[big.tile([128, 128], BF16, name=f"cn{r}i{i}")
                    for i in range(8)] for r in range(2)]
            TRI = big.tile([128, 128], BF16)
            Y0 = big.tile([128, 8, EMB], BF16)
            JNK = big.tile([128, 512], BF16)
            AT = [[atp.tile([128, SPAN], BF16, name=f"at{i}s{s}")
                   for s in range(3)] for i in range(3)]

            scn = [0]

            def sc_tile(shape=None, dtype=F32, tag="sc"):
                scn[0] += 1
                pool = scp if tag == "sc" else ctxp
                return pool.tile(shape or [128, SPAN], dtype, tag=tag,
                                 name=f"{tag}{scn[0]}")

            # ---- warm-up: ramp the PE pstate while DMAs stream ----
            nc.gpsimd.memset(JNK[:], 1.0)
            dmy = sc_tile([128, 512], F32, tag="tp")
            for _ in range(26):
                nc.tensor.matmul(dmy[:], JNK[:, 0:128], JNK[:],
                                 start=True, stop=True)

            # ---- DMA in (only what gates the first scores) ----
            nc.sync.dma_start(WQt[0][:], wq_d[0])
            nc.sync.dma_start(WKt[0][:], wk_d[0])
            nc.sync.dma_start(PSH[:], ps_d[:])
            for kb in range(8):
                nc.sync.dma_start(XTk[kb][:], xT_d[kb, :, :])
            for r in range(2):
                nc.gpsimd.memset(VAr[r][:, :, :, 16:17], 1.0)

            # ---- t0 projections: Qic0 + Qic1 + Kic0 streams, then Kic1 ----
            pq0 = sc_tile()[:, 0:512]
            pq1 = sc_tile()[:, 0:512]
            pk0 = sc_tile([128, 512], F32, tag="ctx")[:]
            for kb in range(8):
                nc.tensor.matmul(pq0, WQt[0][:, kb, :], XTk[kb][:, 0:512],
                                 start=(kb == 0), stop=(kb == 7))
                nc.tensor.matmul(pq1, WQt[0][:, kb, :], XTk[kb][:, 512:1024],
                                 start=(kb == 0), stop=(kb == 7))
                nc.tensor.matmul(pk0, WKt[0][:, kb, :], XTk[kb][:, 0:512],
                                 start=(kb == 0), stop=(kb == 7))
            nc.vector.tensor_copy(QTt[0][:, 0:512], pq0)
            nc.scalar.copy(KTt[0][:, 0:512], pk0)
            nc.vector.tensor_copy(QTt[0][:, 512:1024], pq1)

            def pk1_unit():
                p = sc_tile([128, 512], F32, tag="tp")[:]
                for kb in range(8):
                    nc.tensor.matmul(p, WKt[0][:, kb, :],
                                     XTk[kb][:, 512:1024],
                                     start=(kb == 0), stop=(kb == 7))
                nc.vector.tensor_copy(KTt[0][:, 512:1024], p)
                shift_one(KTt[0], KSt[0], 1)

            def shift_one(T, Ts, ic):
                # shift rows down 16 partitions via a one-hot PE matmul
                p = sc_tile([128, 512], F32, tag="tp")[:]
                nc.tensor.matmul(p, PSH[:], T[:, 512 * ic : 512 * ic + 512],
                                 start=True, stop=True)
                if ic == 0:
                    nc.vector.tensor_copy(Ts[:, 0:512], p)
                else:
                    nc.scalar.copy(Ts[:, 512:1024], p)

            def shift_copy(T, Ts):
                shift_one(T, Ts, 0)
                shift_one(T, Ts, 1)

            shift_copy(QTt[0], QSt[0])
            shift_one(KTt[0], KSt[0], 0)

            # ---- remaining input DMAs (needed only by deferred units) ----
            nc.sync.dma_start(WV[:], wv_d[:].rearrange("k p n -> p k n"))
            nc.sync.dma_start(WQt[1][:], wq_d[1])
            nc.sync.dma_start(WKt[1][:], wk_d[1])
            nc.sync.dma_start(WO[:], wo_d[:].rearrange("r p n -> p r n"))
            nc.sync.dma_start(IDT[:], id_d[:])
            nc.sync.dma_start(TRI[:], tri_d[:])

            # ---- deferred unit queue (popped one per exp flush) ----
            def proj_qk_unit(Wt, T, ic):
                def f():
                    p = sc_tile([128, 512], F32, tag="tp")[:]
                    for kb in range(8):
                        nc.tensor.matmul(
                            p, Wt[:, kb, :],
                            XTk[kb][:, 512 * ic : 512 * ic + 512],
                            start=(kb == 0), stop=(kb == 7))
                    nc.vector.tensor_copy(T[:, 512 * ic : 512 * ic + 512], p)
                return f

            def proj_v_unit(mt, half):
                def f():
                    p = sc_tile([128, 512], F32, tag="tp")[:, 0:128]
                    for kb in range(8):
                        nc.tensor.matmul(
                            p, XTk[kb][:, 128 * mt : 128 * mt + 128],
                            WV[:, kb, 128 * half : 128 * half + 128],
                            start=(kb == 0), stop=(kb == 7))
                    nc.vector.tensor_copy(
                        VAr[half][:, mt, :, 0:16],
                        p.rearrange("p (h e) -> p h e", e=16))
                return f

            def transpose_unit(rb):
                def f():
                    TPb = sc_tile([128, 8, 128], BF16, tag="tp")
                    for ib in range(8):
                        nc.tensor.transpose(
                            TPb[:, ib, :], CTr[rb][:, ib, :], IDT[:])
                        nc.vector.tensor_copy(CNr[rb][ib][:], TPb[:, ib, :])
                return f

            def pass0_unit(ib, ic):
                def f():
                    yp = sc_tile([128, 512], F32, tag="tp")[:]
                    nc.tensor.matmul(
                        yp, CNr[0][ib][:],
                        WO[:, 0, 512 * ic : 512 * ic + 512],
                        start=True, stop=True)
                    nc.vector.tensor_copy(
                        Y0[:, ib, 512 * ic : 512 * ic + 512], yp)
                    nc.sync.dma_start(
                        y0_d[ib, :, 512 * ic : 512 * ic + 512],
                        Y0[:, ib, 512 * ic : 512 * ic + 512])
                return f

            units = [(0, pk1_unit)]  # (min_head, thunk)
            for mt in range(8):
                units.append((0, proj_v_unit(mt, 0)))
            units.append((2, proj_qk_unit(WQt[1], QTt[1], 0)))
            units.append((2, proj_qk_unit(WQt[1], QTt[1], 1)))
            units.append((3, lambda: shift_copy(QTt[1], QSt[1])))
            units.append((3, proj_qk_unit(WKt[1], KTt[1], 0)))
            units.append((3, proj_qk_unit(WKt[1], KTt[1], 1)))
            units.append((4, lambda: shift_copy(KTt[1], KSt[1])))
            for mt in range(8):
                units.append((4, proj_v_unit(mt, 1)))
            units.append((10, transpose_unit(0)))
            for ib in range(8):
                for ic in range(2):
                    units.append((10, pass0_unit(ib, ic)))

            def pop_unit(h):
                # first heads drain 2/flush so VA half-0 completes before ctx(0)
                for _ in range(2 if h < 2 else 1):
                    if units and units[0][0] <= h:
                        units.pop(0)[1]()

            def head_slice(qk, h, lo, size):
                t, hl = divmod(h, 8)
                Tq, Ts = (QTt, QSt) if qk == "q" else (KTt, KSt)
                src = Tq[t] if hl % 2 == 0 else Ts[t]
                base = 16 * (hl - hl % 2)
                return src[base : base + 16, lo : lo + size]

            def head_base(h):
                hl = h % 8
                return 16 * (hl - hl % 2)

            # ---- attention ----
            def emit_mask(A, jb, on_dve=False):
                o = SPAN_OF[jb][1]
                if on_dve:
                    nc.vector.tensor_mul(
                        A[:, o : o + 128], A[:, o : o + 128], TRI[:])
                    return
                nc.gpsimd.affine_select(
                    out=A[:, o : o + 128], in_=A[:, o : o + 128],
                    compare_op=mybir.AluOpType.is_ge,
                    fill=0.0, base=0, pattern=[[1, 128]],
                    channel_multiplier=-1)

            def scores_and_exp(h):
                for s, group in enumerate(SPAN_GROUPS):
                    A = AT[h % 3][s]
                    sct = sc_tile()
                    for jb in group:
                        size = SIZES[jb]
                        i0 = 128 * jb
                        off = SPAN_OF[jb][1]
                        done = 0
                        while done < size:
                            # never cross a 512-f32 PSUM bank boundary
                            cw = min(512 - (off + done) % 512, size - done)
                            nc.tensor.matmul(
                                sct[:, off + done : off + done + cw],
                                head_slice("k", h, i0, 128),
                                head_slice("q", h, i0 + done, cw),
                                start=True, stop=True,
                                tile_position=(head_base(h), 0))
                            done += cw
                    nc.scalar.activation(
                        A[:], sct[:],
                        mybir.ActivationFunctionType.Exp, scale=0.25)
                    last = (h == HPG - 1 and s == 2)
                    for n, jb in enumerate(group):
                        emit_mask(A, jb, on_dve=(last and n % 2 == 0))
                    pop_unit(h)

            def ctx_head(h):
                rb, hh = divmod(h, 8)
                CTX = ctxp.tile([128, 8, 17], F32, tag="ctx", name=f"ctx{h}")
                for ib in range(8):
                    # diag last so ctx starts before this head's masks finish
                    contribs = sorted((jb for jb in range(ib + 1)),
                                      key=lambda jb: ORDER_POS[jb])
                    if ib in contribs:
                        contribs.remove(ib)
                        contribs.append(ib)
                    for idx, jb in enumerate(contribs):
                        s, off = SPAN_OF[jb]
                        o = off + 128 * (ib - jb)
                        nc.tensor.matmul(
                            CTX[:, ib, :], AT[h % 3][s][:, o : o + 128],
                            VAr[rb][:, jb, hh, :],
                            start=(idx == 0), stop=(idx == len(contribs) - 1),
                            skip_group_check=True)
                R3 = yst.tile([128, 8], F32, tag="r3", bufs=2)
                nc.vector.reciprocal(
                    R3[:], CTX[:, :, 16:17].rearrange("p a b -> p (a b)"))
                nc.vector.tensor_mul(
                    CTr[rb][:, :, 16 * hh : 16 * hh + 16],
                    CTX[:, :, 0:16],
                    R3[:].unsqueeze(2).broadcast_to([128, 8, 16]))

            # ---- main loop ----
            for h in range(HPG):
                if h >= 2:
                    ctx_head(h - 2)
                scores_and_exp(h)
            ctx_head(HPG - 2)
            ctx_head(HPG - 1)
            transpose_unit(1)()

            # ---- out-projection pass 1 (heads 8-15 partial) ----
            for ib in range(8):
                yp = sc_tile()[:, 0:1024]
                for ic in range(2):
                    nc.tensor.matmul(
                        yp[:, 512 * ic : 512 * ic + 512],
                        CNr[1][ib][:],
                        WO[:, 1, 512 * ic : 512 * ic + 512],
                        start=True, stop=True)
                Y = yst.tile([128, 1024], BF16, tag="y", bufs=4)
                for ic in range(2):
                    src = yp[:, 512 * ic : 512 * ic + 512]
                    dst = Y[:, 512 * ic : 512 * ic + 512]
                    if ic == 0:
                        nc.vector.tensor_copy(dst, src)
                    else:
                        nc.scalar.copy(dst, src)
                nc.sync.dma_start(y1_d[ib, :, :], Y[:])

    split_excess_waits(nc)
    return nc


_NC_CACHE = None


def _get_nc():
    global _NC_CACHE
    if _NC_CACHE is None:
        _NC_CACHE = build_nc()
    return _NC_CACHE


def _bf(a):
    return np.ascontiguousarray(a).astype(ml_dtypes.bfloat16)


def kernel(x, Wq, Wk, Wv, Wo, bo):
    x = np.asarray(x, dtype=np.float32)
    Wq = np.asarray(Wq, dtype=np.float32)
    Wk = np.asarray(Wk, dtype=np.float32)
    Wv = np.asarray(Wv, dtype=np.float32)
    Wo = np.asarray(Wo, dtype=np.float32)
    bo = np.asarray(bo, dtype=np.float32)

    nc = _get_nc()
    ident = np.eye(128, dtype=np.float32)

    in_maps = []
    for c in range(8):
        b, g = divmod(c, NG)
        cols = slice(GCOLS * g, GCOLS * g + GCOLS)
        wq_c = Wq[:, cols].reshape(8, 128, 2, 128).transpose(2, 1, 0, 3)
        wk_c = Wk[:, cols].reshape(8, 128, 2, 128).transpose(2, 1, 0, 3)
        in_maps.append({
            "xT": _bf(x[b].T.reshape(8, 128, SEQ)),
            "wq": _bf(wq_c),
            "wk": _bf(wk_c),
            "wv": _bf(Wv[:, cols].reshape(8, 128, GCOLS)),
            "wo": _bf(Wo[cols, :].reshape(2, 128, EMB)),
            "ident": _bf(ident),
            "psh": _bf(np.eye(128, k=-16)),
            "tri": _bf(np.triu(np.ones((128, 128), dtype=np.float32))),
        })

    res = run_bass_kernel_spmd(nc, in_maps, core_ids=list(range(8)))
    out = np.zeros((BATCH, SEQ, EMB), dtype=np.float32)
    for c in range(8):
        b = c // NG
        out[b] += res.results[c]["y0"].reshape(SEQ, EMB).astype(np.float32)
        out[b] += res.results[c]["y1"].reshape(SEQ, EMB).astype(np.float32)
    out += bo[None, None, :]
    return out
